# revision 109
# baseline (speedup 1.0000x reference)
"""Trainium2 Bass kernel for multi-head causal attention + output projection.

Problem (hardcoded): B=4, T=2048, E=1024, H=16, D=64, float32.
  q = einsum('bte,hed->bhtd', data, Wq)   (same k, v)
  scores = q@k.T / sqrt(D), causal mask, softmax
  out = (attn @ v) concat-heads @ Wp + bp

Sharding across 8 NeuronCores: core c -> (batch b=c//2, head-group g=c%2).
Each core computes 8 heads (4 "pairs" of 2) of one batch.

Per-core design (all matmul operands bf16; PSUM f32):
 - q/k projections into qT/kT [d-pair(128) x T] in 512-token chunks; v
   into vall [t-in-tile(128) x (t-tile, head, 65)] natural layout with a
   ones column at d=64 per head block.
 - scores kept TRANSPOSED (psS[tk, tq]) so the softmax key-reduction is a
   PE contraction; exp on ACT (no max subtraction: scores ~ N(0,1)).
 - attn@V with the EXP TILE AS STATIONARY and V as moving: output lands
   in natural [tq, d] layout at full PE rate (65-row moving), and the
   ones column delivers sum(exp) as psum col 64 of each accumulator.
 - normalization = per-partition reciprocal + tensor_scalar_mul on DVE
   (no PE broadcast needed in natural layout).
 - attention output transposed back to [c, t] via PE transpose for the
   output projection; Wp projection accumulates over 4 c-tiles.
 - causal: key-tiles above the diagonal skipped; diagonal tiles trim the
   query range to >= o*128 and mask only the 128x128 boundary triangle.
 - scheduling: the Tile scheduler is a priority list scheduler (priority
   = emission order). Attention (scores/exp/attn@V) is foreground;
   q/k/v chunk projections and the output projection are background
   tiers that fill PE slack; a lowest-priority PE warm-up chain keeps
   the p-state ramped through the DMA head. Blocks run query-block-
   outer so each block row's projection spreads over later rows; the
   last row normalizes per sub-tile and projects through freed psS
   slots to shorten the drain tail.

PSUM bank map (2KB x 8):
  banks 0-3: psS double-buffer (2 x [128,1024] f32), also the last
             row's projection accumulators
  banks 4-5: psAVa/psAVb: per-head attn@V accumulators (4 x 65 cols)
  bank  6:   psT: transpose staging (bf16, half-rotated) + chunk psum
  bank  7:   pq: q/k/v chunk projections + inline output projection

Host side: out[b] = core(2b) + core(2b+1) partials, + bias.
"""

import numpy as np

import concourse.bass as bass
import concourse.mybir as mybir
import concourse.tile as tile
from concourse import masks as cmasks
from contextlib import ExitStack

F32 = mybir.dt.float32
BF16 = mybir.dt.bfloat16

# Full-problem constants
B, T, E, H, D = 4, 2048, 1024, 16, 64
N_CORES = 8
H_LOC = H // 2          # heads per core
HP = H_LOC // 2         # head pairs per core
C = H_LOC * D           # local concat width (512)
ET = E // 128           # embedding 128-tiles
TT = T // 128           # token 128-tiles
NJB = 4                 # query blocks of 512
TQB = 512
SCALE = float(D) ** -0.5

# schedule toggles (bisected by experiment)
EE_BUFS = 12            # exp-tile pool depth
WARMUP = 80             # PE warm-up transposes before real work
PROJ_TIER = 2           # 0=foreground, N=background tier for proj
ACT_TEVICT = False      # transpose evictions on ACT instead of DVE
JB_OUTER = True         # query-block-outer vs pair-outer block order
SUBNORM = True          # per-sub normalize in the last row
MASK_GPSIMD = False     # causal boundary masks on GPSIMD instead of DVE
AV_TIER = 0             # background tier for attn@V matmuls (0=foreground)
XC0_SPLIT = True        # split x chunk 0 DMA per e-tile
OT_BUFS = 12            # out-staging pool depth
RR_BUFS = 4             # reciprocal pool depth
V_SEP = False           # all q/k chunks ahead of all V rounds
KE_SPLIT = False        # split the prefix k-chunk eviction per key tile
MASK_LATE = False       # mask DMA after the prefix weight columns
V0_LATE = False         # V chunk-0 rounds behind the other pairs' row-0 qk
DMA_TRANSP = True       # inline-row transposes via DMA xbar instead of PE
PROJ_ALT = False        # inline proj rounds alternate pq/psT banks


def build_program(nc):
    AF = mybir.ActivationFunctionType

    xTd = nc.dram_tensor("xT", [E, T], BF16, kind="ExternalInput").ap()
    wqkvd = nc.dram_tensor("wqkv", [E, 3 * C], BF16,
                           kind="ExternalInput").ap()
    wpd = nc.dram_tensor("wp", [C, E], BF16, kind="ExternalInput").ap()
    maskd = nc.dram_tensor("mask", [128, 128], BF16,
                           kind="ExternalInput").ap()
    outd = nc.dram_tensor("out", [T, E], BF16,
                          kind="ExternalOutput").ap()

    with tile.TileContext(nc) as tc, ExitStack() as ctx:
        sb = ctx.enter_context(tc.tile_pool(name="sb", bufs=1))
        ident = sb.tile([128, 128], BF16, name="ident")
        mask_sb = sb.tile([128, 128], BF16, name="mask_sb")
        warm = sb.tile([128, 128], BF16, name="warm")
        # e-tiles packed along the free dim of single wide tiles so one
        # strided DMA covers a column group across all e
        wqkvt = sb.tile([128, ET * 3 * C], BF16, name="wqkvt")
        xtt = sb.tile([128, ET * T], BF16, name="xtt")

        def wq_ap(e, a, b):
            return wqkvt[:, e * 3 * C + a:e * 3 * C + b]

        def xt_ap(e, a, b):
            return xtt[:, e * T + a:e * T + b]
        qT = [sb.tile([128, T], BF16, name=f"qT{p}") for p in range(HP)]
        kT = [sb.tile([128, T], BF16, name=f"kT{p}") for p in range(HP)]
        # v in natural layout + ones column: col = (t*8 + lh)*65 + d,
        # d=64 is the ones column (yields sum(exp) through the AV matmul)
        vall = sb.tile([128, TT * H_LOC * 65], BF16, name="vall")
        # attention out, natural [tq x (tt, h, d)] per pair
        olt_nat = [sb.tile([128, T], BF16, name=f"oltn{p}") for p in range(HP)]
        # attention out, transposed [c x t], pair c-blocks packed along free
        olt_all = sb.tile([128, HP * T], BF16, name="olt_all")
        wpt = [sb.tile([128, E], BF16, name=f"wpt{p}") for p in range(HP)]

        ee_pool = ctx.enter_context(tc.tile_pool(name="ee", bufs=EE_BUFS))
        rr_pool = ctx.enter_context(tc.tile_pool(name="rr", bufs=RR_BUFS))
        ot_pool = ctx.enter_context(tc.tile_pool(name="ot", bufs=OT_BUFS))

        ps_pool = ctx.enter_context(
            tc.tile_pool(name="ps", bufs=1, space="PSUM"))
        psS_pool = ctx.enter_context(
            tc.tile_pool(name="pss", bufs=2, space="PSUM"))
        # per-head AV accumulators: 4 x 65 cols each, one bank per head
        psAVa = ps_pool.tile([128, 260], F32, name="psAVa")
        psAVb = ps_pool.tile([128, 260], F32, name="psAVb")
        psT = ps_pool.tile([128, 512], F32, name="psT")
        pq = ps_pool.tile([128, 512], F32, name="pqbank")
        psT_bf = psT[:].bitcast(BF16)  # [128, 1024] bf16 (8 x 128 regions)

        cmasks.make_identity(nc, ident[:])
        nc.gpsimd.memset(vall[:, 64::65], 1.0)  # ones columns
        if not MASK_LATE:
            nc.sync.dma_start(mask_sb[:], maskd)

        def dma_wcols(a, b):
            # wqkv cols [a:b) for ALL e in one strided transfer
            w = b - a
            src = bass.AP(wqkvd.tensor, a,
                          [[3 * C, 128], [3 * C * 128, ET], [1, w]])
            dst = bass.AP(wqkvt.tensor, wqkvt.offset + a,
                          [list(wqkvt.ap[0]), [3 * C, ET], [1, w]])
            nc.sync.dma_start(dst, src)

        def dma_xchunk(cch):
            # x token-chunk for ALL e in one strided transfer
            src = bass.AP(xTd.tensor, cch * 512,
                          [[T, 128], [T * 128, ET], [1, 512]])
            dst = bass.AP(xtt.tensor, xtt.offset + cch * 512,
                          [list(xtt.ap[0]), [T, ET], [1, 512]])
            nc.sync.dma_start(dst, src)

        # priority order: pair-0 q/k columns and x chunk 0 first (first
        # attention block starts ~8us in), then the remaining weight
        # columns so row-0 chunks of pairs 1-3 can fill the head window,
        # then the remaining x chunks
        dma_wcols(0, 128)            # q cols, pair 0
        dma_wcols(C, C + 128)        # k cols, pair 0
        if MASK_LATE:
            nc.sync.dma_start(mask_sb[:], maskd)
        if XC0_SPLIT:
            # per-e transfers so the first chunk's e-accumulation matmuls
            # pipeline with the arriving data
            for e in range(ET):
                nc.sync.dma_start(xt_ap(e, 0, 512),
                                  xTd[e * 128:(e + 1) * 128, 0:512])
        else:
            dma_xchunk(0)
        dma_wcols(2 * C, 3 * C)      # all v cols
        dma_wcols(128, C)            # q cols, pairs 1-3
        dma_wcols(C + 128, 2 * C)    # k cols, pairs 1-3
        for cch in range(1, 4):
            dma_xchunk(cch)
        for p in range(HP):
            nc.sync.dma_start(wpt[p][:], wpd[p * 128:(p + 1) * 128, :])

        # The Tile scheduler is a priority list scheduler (priority =
        # emission order): each engine pops the highest-priority READY
        # instruction. So we emit ALL work up front in natural dependency
        # order and let the scheduler pack the engines; no manual pacing.

        # chunk projections double-buffer across the pq and psT banks so the
        # DVE eviction of one chunk overlaps the next chunk's matmuls
        chunk_bank = [0]

        def next_bank():
            chunk_bank[0] ^= 1
            return pq if chunk_bank[0] else psT

        def emit_v_round(t):
            # all 8 heads of one token-tile: psv [128t x 512(lh,d)]
            bank = next_bank()
            for e in range(ET):
                nc.tensor.matmul(
                    bank[:, 0:512],
                    xt_ap(e, t * 128, (t + 1) * 128),
                    wq_ap(e, 2 * C, 3 * C),
                    start=(e == 0), stop=(e == ET - 1))
            # scatter into vall's 65-wide head blocks (ones col untouched)
            src = bass.AP(bank.tensor, bank.offset,
                          [list(bank.ap[0]), [64, 8], [1, 64]])
            dst = bass.AP(vall.tensor, vall.offset + t * 520,
                          [list(vall.ap[0]), [65, 8], [1, 64]])
            nc.vector.tensor_copy(dst, src)

        def emit_qk_chunk(dst, p, cch, woff, split_evict=False):
            bank = next_bank()
            for e in range(ET):
                nc.tensor.matmul(
                    bank[:, 0:512],
                    wq_ap(e, woff + p * 128, woff + (p + 1) * 128),
                    xt_ap(e, cch * 512, (cch + 1) * 512),
                    start=(e == 0), stop=(e == ET - 1))
            if split_evict:
                # per-key-tile pieces so the first scores matmul can start
                # after the first piece (prefix critical path only)
                for i in range(4):
                    nc.vector.tensor_copy(
                        dst[p][:, cch * 512 + i * 128:
                               cch * 512 + (i + 1) * 128],
                        bank[:, i * 128:(i + 1) * 128])
            else:
                nc.vector.tensor_copy(
                    dst[p][:, cch * 512:(cch + 1) * 512], bank[:, 0:512])

        # ---------------- output projection work -------------------------
        def emit_transp(tt):
            half = (tt % 2) * 512
            for p4 in range(HP):
                nc.tensor.transpose(
                    psT_bf[:, half + p4 * 128:half + (p4 + 1) * 128],
                    olt_nat[p4][:, tt * 128:(tt + 1) * 128],
                    ident[:])

        def emit_transp_evict(tt, late=False):
            # one strided copy for all 4 pair c-blocks (GPSIMD cannot
            # access PSUM, so DVE/ACT)
            half = (tt % 2) * 512
            base = psT_bf[:, half:half + 512]
            src = bass.AP(base.tensor, base.offset,
                          [list(base.ap[0]), [128, 4], [1, 128]])
            dst = bass.AP(olt_all.tensor, olt_all.offset + tt * 128,
                          [list(olt_all.ap[0]), [T, 4], [1, 128]])
            eng = nc.scalar.copy if (ACT_TEVICT or late) else \
                nc.vector.tensor_copy
            eng(dst, src)

        def emit_proj_mm(tt, ec, bank):
            for p4 in range(HP):
                nc.tensor.matmul(
                    bank[:, 0:512],
                    olt_all[:, p4 * T + tt * 128:p4 * T + (tt + 1) * 128],
                    wpt[p4][:, ec * 512:(ec + 1) * 512],
                    start=(p4 == 0), stop=(p4 == HP - 1))

        def emit_proj_evict(tt, ec, ot, bank):
            nc.vector.tensor_copy(ot[:, ec * 512:(ec + 1) * 512],
                                  bank[:, 0:512])

        def emit_proj(jb, late=False):
            for tt in range(4 * jb, 4 * jb + 4):
                if DMA_TRANSP and (DMA_TRANSP == 2 or not late):
                    # inline rows tolerate the higher DMA-transpose latency;
                    # this moves the transpose off PE and its eviction off
                    # DVE onto the otherwise-idle DMA engines
                    for p4 in range(HP):
                        nc.sync.dma_start_transpose(
                            olt_all[:, p4 * T + tt * 128:
                                    p4 * T + (tt + 1) * 128],
                            olt_nat[p4][:, tt * 128:(tt + 1) * 128])
                else:
                    emit_transp(tt)
                    emit_transp_evict(tt, late)
                if late:
                    # after the last exp, psS slots and ACT are free: both
                    # e-chunks into one rotating [128,1024] slot, one ACT
                    # eviction, halves per-tile DMA count
                    psp = psS_pool.tile([128, 1024], F32, tag="s",
                                        name="psp")
                    ot = ot_pool.tile([128, E], BF16, tag="ot", name="ot")
                    for ec in range(2):
                        for p4 in range(HP):
                            nc.tensor.matmul(
                                psp[:, ec * 512:(ec + 1) * 512],
                                olt_all[:, p4 * T + tt * 128:
                                        p4 * T + (tt + 1) * 128],
                                wpt[p4][:, ec * 512:(ec + 1) * 512],
                                start=(p4 == 0), stop=(p4 == HP - 1))
                    nc.scalar.copy(ot[:], psp[:])
                    nc.sync.dma_start(outd[tt * 128:(tt + 1) * 128, :],
                                      ot[:])
                else:
                    ot = ot_pool.tile([128, E], BF16, tag="ot", name="ot")
                    if PROJ_ALT:
                        # e-chunk rounds on alternating banks so their
                        # evictions overlap the other round's matmuls
                        b0, b1 = (pq, psT) if tt % 2 == 0 else (psT, pq)
                        emit_proj_mm(tt, 0, b0)
                        emit_proj_evict(tt, 0, ot, b0)
                        emit_proj_mm(tt, 1, b1)
                        emit_proj_evict(tt, 1, ot, b1)
                    else:
                        emit_proj_mm(tt, 0, pq)
                        emit_proj_evict(tt, 0, ot, pq)
                        emit_proj_mm(tt, 1, pq)
                        emit_proj_evict(tt, 1, ot, pq)
                    nc.sync.dma_start(outd[tt * 128:(tt + 1) * 128, :],
                                      ot[:])

        # ---------------- attention --------------------------------------
        def emit_attn_block(p, jb, subnorm=False):
            n_tk = 4 * jb + 4
            psAV = (psAVa, psAVb)

            def normalize(h, s, rr, ri):
                tt = 4 * jb + s
                nc.vector.tensor_scalar_mul(
                    olt_nat[p][:, tt * 128 + h * 64:tt * 128 +
                               (h + 1) * 64],
                    psAV[h][:, s * 65:s * 65 + 64],
                    rr[:, ri:ri + 1])

            for tk in range(n_tk):
                o = tk - 4 * jb
                sh = max(0, o)      # causal trim: queries < sh*128 skipped
                lo = sh * 128
                ksl = slice(tk * 128, (tk + 1) * 128)
                qsl = slice(jb * TQB + lo, (jb + 1) * TQB)
                psS = psS_pool.tile([128, 1024], F32, tag="s", name="psS")
                nc.tensor.matmul(psS[:, lo:512],
                                 kT[p][0:64, ksl], qT[p][0:64, qsl],
                                 start=True, stop=True)
                nc.tensor.matmul(psS[:, 512 + lo:1024],
                                 kT[p][64:128, ksl], qT[p][64:128, qsl],
                                 start=True, stop=True)
                ee = ee_pool.tile([128, 1024], BF16, tag="ee", name="ee")
                nc.scalar.activation(ee[:, lo:1024], psS[:, lo:1024],
                                     AF.Exp, scale=SCALE)
                if o >= 0:
                    meng = nc.gpsimd if MASK_GPSIMD else nc.vector
                    for h in range(2):
                        r = slice(h * 512 + o * 128, h * 512 + (o + 1) * 128)
                        meng.tensor_mul(ee[:, r], ee[:, r], mask_sb[:])
                # attn@V: start=True zeroes/marks the WHOLE 2KB psum bank:
                # exactly one start (first matmul of the block into the
                # bank) and one stop (last); the pending-zero mechanism
                # zeroes each 65-col region at first write.
                def emit_av(tk=tk, sh=sh, ee=ee):
                    for h in range(2):
                        for s in range(sh, 4):
                            st = ee[:, h * 512 + s * 128:
                                    h * 512 + (s + 1) * 128]
                            first = tk == 0 and s == sh
                            last = tk == n_tk - 1 and s == 3
                            nc.tensor.matmul(
                                psAV[h][:, s * 65:(s + 1) * 65],
                                st,
                                vall[:, (tk * 8 + 2 * p + h) * 65:
                                     (tk * 8 + 2 * p + h) * 65 + 65],
                                start=first, stop=last,
                                skip_group_check=True)
                if AV_TIER:
                    with background(AV_TIER):
                        emit_av()
                else:
                    emit_av()
                if subnorm and o >= 0:
                    # sub o's accumulation just finished: normalize now so
                    # the projection's transposes can start before the
                    # whole block completes (shortens the drain tail)
                    for h in range(2):
                        rr = rr_pool.tile([128, 1], F32, tag="rr1",
                                          name="rr")
                        nc.vector.reciprocal(
                            rr[:], psAV[h][:, o * 65 + 64:o * 65 + 65])
                        normalize(h, o, rr, 0)

            if not subnorm:
                # normalize: 1/sumexp per tq partition (col 64 of each
                # region), scale psAV into olt_nat
                for h in range(2):
                    rr = rr_pool.tile([128, 4], F32, tag="rr", name="rr")
                    nc.vector.reciprocal(rr[:], psAV[h][:, 64::65])
                    for s in range(4):
                        normalize(h, s, rr, s)

        # ---------------- schedule ---------------------------------------
        from contextlib import contextmanager

        @contextmanager
        def background(tier=1):
            # Low scheduler priority: these instructions only run when the
            # engine has no ready foreground (attention-critical) work.
            orig = tc.cur_priority
            tc.cur_priority = orig + tier * 1_000_000
            try:
                yield
            finally:
                tc.cur_priority = orig

        # PE warm-up: keep the tensor engine busy (and its p-state ramped)
        # through the DMA head. Lowest priority: real work preempts the
        # chain the moment it becomes ready, with no PE idle in between.
        with background(tier=3):
            for i in range(WARMUP):
                nc.tensor.transpose(psT_bf[:, 0:128], ident[:], ident[:])
            if WARMUP:
                nc.scalar.copy(warm[:], psT_bf[:, 0:128])

        # prefix: ONLY pair-0 q/k at top priority (unblocks scores/exp
        # ASAP); everything else -- including V c0, which only the (lag-
        # tolerant) attn@V consumes -- fills PE slack from the background
        emit_qk_chunk(qT, 0, 0, 0)
        emit_qk_chunk(kT, 0, 0, C, split_evict=KE_SPLIT)
        if V_SEP:
            # all q/k chunks ahead of all V rounds (attn@V tolerates V
            # arriving late through the ee-pool lag)
            with background():
                for t in range(4):
                    emit_v_round(t)
                for p in range(1, HP):
                    emit_qk_chunk(qT, p, 0, 0)
                    emit_qk_chunk(kT, p, 0, C)
                for cch in range(1, NJB):
                    for p in range(HP):
                        emit_qk_chunk(qT, p, cch, 0)
                        emit_qk_chunk(kT, p, cch, C)
            with background(tier=2):
                for cch in range(1, NJB):
                    for t in range(4 * cch, 4 * cch + 4):
                        emit_v_round(t)
        else:
            with background():
                if not V0_LATE:
                    for t in range(4):
                        emit_v_round(t)
                for p in range(1, HP):
                    emit_qk_chunk(qT, p, 0, 0)
                    emit_qk_chunk(kT, p, 0, C)
                if V0_LATE:
                    for t in range(4):
                        emit_v_round(t)
                for cch in range(1, NJB):
                    for p in range(HP):
                        emit_qk_chunk(qT, p, cch, 0)
                        emit_qk_chunk(kT, p, cch, C)
                    for t in range(4 * cch, 4 * cch + 4):
                        emit_v_round(t)

        def proj_wrapped(jb, late=False):
            if PROJ_TIER:
                with background(tier=PROJ_TIER):
                    emit_proj(jb, late)
            else:
                emit_proj(jb, late)

        if JB_OUTER == 2:
            # hybrid: all of pair 0 first (a long early exp stream fed by
            # only the prefix chunks), then jb-outer rows of pairs 1-3
            # with each row's projection inline
            for jb in range(NJB):
                emit_attn_block(0, jb)
            for jb in range(NJB):
                for p in range(1, HP):
                    emit_attn_block(
                        p, jb, subnorm=(SUBNORM == 2 or
                                        (SUBNORM and jb == NJB - 1)))
                proj_wrapped(jb, late=(jb == NJB - 1))
        elif JB_OUTER:
            # each query block's projection becomes ready right after its
            # row of pair blocks, so projection work spreads across the
            # whole attention span and the tail is only the last jb's
            for jb in range(NJB):
                for p in range(HP):
                    emit_attn_block(
                        p, jb, subnorm=(SUBNORM == 2 or
                                        (SUBNORM and jb == NJB - 1)))
                proj_wrapped(jb, late=(jb == NJB - 1))
        else:
            for p in range(HP - 1):
                for jb in range(NJB):
                    emit_attn_block(p, jb)
            for jb in range(NJB):
                emit_attn_block(HP - 1, jb)
                proj_wrapped(jb)
    return nc


# ---------------- host side ----------------------------------------------

def _bf(a):
    import ml_dtypes
    return np.ascontiguousarray(a).astype(ml_dtypes.bfloat16)


def make_mask():
    import ml_dtypes
    tk = np.arange(128)[:, None]
    tq = np.arange(128)[None, :]
    return (tq >= tk).astype(ml_dtypes.bfloat16)


def shard_inputs(data, Wq, Wk, Wv, Wp):
    """Build the 8 per-core input maps from full inputs."""
    data = np.asarray(data, np.float32)
    Wq = np.asarray(Wq, np.float32)
    Wk = np.asarray(Wk, np.float32)
    Wv = np.asarray(Wv, np.float32)
    Wp = np.asarray(Wp, np.float32)
    mask = make_mask()
    in_maps = []
    for c in range(N_CORES):
        b, g = c // 2, c % 2
        hs = slice(g * H_LOC, (g + 1) * H_LOC)
        wq = Wq[hs].transpose(1, 0, 2).reshape(E, C)
        wk = Wk[hs].transpose(1, 0, 2).reshape(E, C)
        wv = Wv[hs].transpose(1, 0, 2).reshape(E, C)
        in_maps.append({
            "xT": _bf(data[b].T),
            "wqkv": _bf(np.concatenate([wq, wk, wv], axis=1)),
            "wp": _bf(Wp[g * C:(g + 1) * C, :]),
            "mask": mask,
        })
    return in_maps


_NC_CACHE = {}


def legalize_single_wait(nc):
    """This toolchain's walrus accepts at most ONE sync wait per engine
    instruction; Tile freely emits more. Split extra waits onto preceding
    same-engine NoOps (engine FIFOs make that equivalent)."""
    import bass_rust
    cnt = 0
    for f in nc.m.functions:
        for blk in f.blocks:
            new = []
            changed = False
            for inst in blk.instructions:
                si = inst.sync_info
                if si is not None and len(si.on_wait) > 1:
                    waits = list(si.on_wait)
                    for w in waits[:-1]:
                        nop = bass_rust.InstNoOp(name=f"legal_nop_{cnt}")
                        cnt += 1
                        nop.engine = inst.engine
                        nop.sync_info = bass_rust.SyncInfo(on_wait=[w],
                                                           on_update=[])
                        new.append(nop)
                    inst.sync_info = bass_rust.SyncInfo(
                        on_wait=[waits[-1]], on_update=list(si.on_update))
                    changed = True
                new.append(inst)
            if changed:
                blk.instructions = new
    return cnt


def get_nc():
    if "nc" not in _NC_CACHE:
        nc = bass.Bass("TRN2", target_bir_lowering=False, debug=False,
                       num_devices=N_CORES)
        build_program(nc)
        legalize_single_wait(nc)
        _NC_CACHE["nc"] = nc
    return _NC_CACHE["nc"]


def run(inputs, trace=False, **kw):
    """Run on the 8 NeuronCores; returns (full_output, BassKernelResults)."""
    from concourse.bass_utils import run_bass_kernel_spmd
    nc = get_nc()
    in_maps = shard_inputs(inputs["data"], inputs["Wq"], inputs["Wk"],
                           inputs["Wv"], inputs["Wp"])
    res = run_bass_kernel_spmd(nc, in_maps, core_ids=list(range(N_CORES)),
                               trace=trace, **kw)
    bp = np.asarray(inputs["bp"], np.float32)
    outf = np.empty((B, T, E), np.float32)
    for b in range(B):
        outf[b] = (res.results[2 * b]["out"].astype(np.float32)
                   + res.results[2 * b + 1]["out"].astype(np.float32) + bp)
    return outf, res


def kernel(**inputs):
    out, _ = run(inputs)
    return out
